# revision 21
# baseline (speedup 1.0000x reference)
"""Trainium2 Bass kernel for the ContinuousVariableQNN problem.

Math reduction (validated against the jax reference on host):
  The reference builds a 256x256 symplectic matrix S from params, then
    mu   = mu0 @ S.T   with mu0[:, 0::2] = 2*inputs (odd cols zero)
    n    = (dsum + mu_x^2 + mu_p^2) / (2*hbar) - 0.5
  Because mu0's p-quadrature entries are all zero, the big matmul collapses to
    mu_dev = inputs @ Ms          with Ms[i, j] = S[j, 2*i]   ([128, 256])
  and n[b, m] = mu_x[b, m]^2 + mu_p[b, m]^2 + bias[m]; bias is host-side.

Device strategy (pure data parallelism over 8 cores, batch-sharded), v4:
  Transposed world: host pre-casts X to fp16 and pre-transposes each core's
  shard to XT [128 feat, 16384 rows]; no on-device transpose.  Per core,
  16 groups of 1024 batch rows:
    4 matmuls per group with Ms halves as the bf16 STATIONARY operand and
    XT slices as the fp16 MOVING operand (512 rows each):
      mu [128 modes, 4, 512] = [x0 x1 p0 p1]  (f32 PSUM, 4 banks, 2 bufs)
    ACT squares the x-half (PSUM -> SBUF bf16); a custom DVE uop
    (out = in0^2 + in1 + s0) reads the p-half from PSUM once (the BIR
    verifier allows only ONE PSUM operand per DVE instruction), squares,
    adds the x-square, all in one pass.  2 of 16 groups instead let ACT
    square both halves and DVE do a cheap bf16 2x add, balancing engines.
  Output nT [128 modes, 16384 rows] bf16 DMAs per-chunk on the GPSIMD
  SWDGE queue (ACT HWDGE queue stays free for compute); input chunks ride
  the SP HWDGE queue with a graduated first chunk.  Host adds bias and
  un-transposes.  HBM: 4.2 MB in (fp16) + 4.2 MB out (bf16) per core.
  Measured end-to-end max rel err ~1.6e-2 (gate 2e-2; inputs deterministic).
"""

import ml_dtypes
import numpy as np

import concourse.bass as bass
import concourse.mybir as mybir
import concourse.tile as tile
from concourse import bacc
from concourse import dve_ops as _dve_ops
from concourse.bass_utils import run_bass_kernel_spmd
from concourse.dve_spec import C0, Spec, Src0, Src1
from concourse.dve_spec import _has_src1
from concourse.dve_spec import lower as _dve_lower
from concourse.dve_spec import sq as _sq
from concourse.dve_uop import DveOpSpec

# ---- custom DVE op: out = in0^2 + in1 + s0 --------------------------------
# Single-source square (one PSUM read) fused with the SBUF add.
_SQADD_NAME = "SQUARE_ADD_BIAS_ANT"


def _install_sqadd_op() -> "_dve_ops.DveOp":
    for op in _dve_ops.OPS:
        if op.name == _SQADD_NAME:
            return op
    spec = Spec(
        body=_sq(Src0) + Src1 + C0,
        reference=lambda in0, in1, s0, s1, imm2: (
            in0.astype(np.float32) ** 2 + in1 + s0),
    )
    row = _dve_ops._CUSTOM_DVE_ROW_BASE + len(_dve_ops.OPS)
    assert row < 0x20
    _dve_ops._SUB_OPCODE_FOR_NAME[_SQADD_NAME] = row
    shas = {}
    for ver in ("v3", "v4"):
        s = DveOpSpec(name=_SQADD_NAME, opcode=row,
                      uops=_dve_lower(spec, ver=ver), rd1_en=_has_src1(spec))
        shas[ver] = s.sha(ver)
    op = _dve_ops.DveOp(_SQADD_NAME, spec, subdim=False, uops_sha=shas)
    _dve_ops.OPS.append(op)
    _dve_ops.CUSTOM_DVE_SPECS[_SQADD_NAME] = spec
    return op


_SQADD_OP = _install_sqadd_op()

N_QUMODES = 128
N_LAYERS = 8
BATCH = 131072
N_CORES = 8
ROWS = BATCH // N_CORES          # 16384 rows per core
CHUNK = 2048                     # batch rows per chunk
N_CHUNKS = ROWS // CHUNK         # 8
GROUP = 1024                     # batch rows per compute group
N_GROUPS = ROWS // GROUP         # 16
F32 = mybir.dt.float32
F16 = mybir.dt.float16
BF16 = mybir.dt.bfloat16

# Group schedule: False (type A) -> ACT squares x-half, DVE runs the fused
# p^2+add op.  True (type C) -> ACT squares both halves, DVE does the cheap
# all-SBUF bf16 2x add.  14 A / 2 C balances ACT vs DVE.
GROUP_TYPE_C = [False, False, False, False, False, True, False, False,
                False, False, False, False, False, True, False, False]


def host_prep(params: np.ndarray):
    """Build Msx/Msp [128, 128] bf16 and bias [128] f64 on host (tiny)."""
    L, N = N_LAYERS, N_QUMODES
    p = params.reshape(L, N, 3).astype(np.float64)
    th1, r, th2 = p[..., 0], p[..., 1], p[..., 2]

    def rot(th):
        c, s = np.cos(th), np.sin(th)
        return np.stack([np.stack([c, -s], -1), np.stack([s, c], -1)], -2)

    z = np.zeros_like(r)
    sqz = np.stack([np.stack([np.exp(-r), z], -1),
                    np.stack([z, np.exp(r)], -1)], -2)
    blk = np.einsum('lnab,lnbc,lncd->lnad', rot(th2), sqz, rot(th1))

    t = np.cos(np.pi / 4)
    rr = np.sin(np.pi / 4)
    BS4 = np.array([[t, 0., -rr, 0.],
                    [0., t, 0., -rr],
                    [rr, 0., t, 0.],
                    [0., rr, 0., t]], dtype=np.float64)
    C = np.eye(2 * N, dtype=np.float64)
    for i in range(N - 1):
        C[2 * i:2 * i + 4, :] = BS4 @ C[2 * i:2 * i + 4, :]

    S = np.eye(2 * N, dtype=np.float64)
    idx = np.arange(N)
    for l in range(L):
        D = np.zeros((N, 2, N, 2), np.float64)
        D[idx, :, idx, :] = blk[l]
        S = C @ (D.reshape(2 * N, 2 * N) @ S)

    Ms = S[:, 0::2].T                                        # [128 feat, 256]
    Ms_xp = np.ascontiguousarray(
        np.concatenate([Ms[:, 0::2], Ms[:, 1::2]], axis=1),
        dtype=ml_dtypes.bfloat16)                            # [x | p]

    dV = (S ** 2).sum(axis=1)                                # [256]
    bias = (dV[0::2] + dV[1::2]) / 4.0 - 0.5                 # [128] f64
    return Ms_xp, bias


def build_bass():
    nc = bacc.Bacc("TRN2", target_bir_lowering=False, debug=False,
                   num_devices=N_CORES)

    xt_d = nc.dram_tensor("xt", [128, ROWS], F16, kind="ExternalInput")
    ms_d = nc.dram_tensor("ms", [128, 256], BF16, kind="ExternalInput")
    out_d = nc.dram_tensor("outT", [128, ROWS], BF16, kind="ExternalOutput")

    with tile.TileContext(nc) as tc:
        with (
            tc.tile_pool(name="const", bufs=1) as const_pool,
            tc.tile_pool(name="xin", bufs=N_CHUNKS) as xin_pool,
            tc.tile_pool(name="sq", bufs=3) as sq_pool,
            tc.tile_pool(name="oout", bufs=3) as oout_pool,
            tc.tile_pool(name="mu", bufs=2, space="PSUM") as mu_pool,
        ):
            # First 512 input columns, then the (tiny) Ms constant, both on
            # the SP HWDGE queue so the PE can start ASAP.  Input chunks are
            # split across the SP and ACT HWDGE queues (outputs ride the
            # GPSIMD SWDGE queue, so the ACT ring is otherwise free).
            xt_tiles = [
                xin_pool.tile([128, CHUNK], F16, tag="xt", name=f"xt_{c}")
                for c in range(N_CHUNKS)
            ]
            nc.sync.dma_start(out=xt_tiles[0][:, 0:512],
                              in_=xt_d.ap()[:, 0:512])
            ms_sb = const_pool.tile([128, 256], BF16)
            nc.sync.dma_start(out=ms_sb, in_=ms_d.ap())
            msx_sb = ms_sb[:, 0:128]
            msp_sb = ms_sb[:, 128:256]
            nc.sync.dma_start(out=xt_tiles[0][:, 512:1024],
                              in_=xt_d.ap()[:, 512:1024])
            nc.scalar.dma_start(out=xt_tiles[0][:, 1024:2048],
                                in_=xt_d.ap()[:, 1024:2048])
            for c in range(1, N_CHUNKS):
                eng = nc.sync if c % 2 == 1 else nc.scalar
                eng.dma_start(out=xt_tiles[c],
                              in_=xt_d.ap()[:, c * CHUNK:(c + 1) * CHUNK])

            ot_tiles = {}
            for g in range(N_GROUPS):
                c, half = divmod(g, 2)
                if half == 0:
                    ot_tiles[c] = oout_pool.tile([128, 2, GROUP], BF16,
                                                 tag="ot", name=f"ot_{c}")
                x_sb = xt_tiles[c]
                # mu layout per group: [x0 | x1 | p0 | p1], 512 rows each
                mu_ps = mu_pool.tile([128, 4, 512], F32, tag="mu",
                                     name=f"mu_{g}")         # 4 PSUM banks
                for h in range(2):
                    rhs = x_sb[:, half * GROUP + h * 512:
                               half * GROUP + (h + 1) * 512]
                    nc.tensor.matmul(mu_ps[:, h, :], msx_sb, rhs,
                                     start=True, stop=True)
                for h in range(2):
                    rhs = x_sb[:, half * GROUP + h * 512:
                               half * GROUP + (h + 1) * 512]
                    nc.tensor.matmul(mu_ps[:, 2 + h, :], msp_sb, rhs,
                                     start=True, stop=True)

                sq_sb = sq_pool.tile([128, 2, GROUP], BF16, tag="sq",
                                     name=f"sq_{g}")         # [x(1024)|p(1024)]
                mux = mu_ps[:, 0:2, :].rearrange("p a b -> p (a b)")
                mup = mu_ps[:, 2:4, :].rearrange("p a b -> p (a b)")
                sqx = sq_sb[:, 0, :]
                n_out = ot_tiles[c][:, half, :]
                if GROUP_TYPE_C[g]:
                    nc.scalar.activation(
                        sq_sb.rearrange("p a b -> p (a b)"),
                        mu_ps.rearrange("p a b -> p (a b)"),
                        mybir.ActivationFunctionType.Square)
                    nc.vector.tensor_tensor(out=n_out, in0=sqx,
                                            in1=sq_sb[:, 1, :],
                                            op=mybir.AluOpType.add)
                else:
                    nc.scalar.activation(sqx, mux,
                                         mybir.ActivationFunctionType.Square)
                    nc.vector._custom_dve(
                        _SQADD_OP, out=n_out, in0=mup, in1=sqx, s0=0.0)
                # Per-group output DMA (2 KB/partition, still full-rate)
                # keeps the drain after the last matmul short.
                nc.gpsimd.dma_start(
                    out=out_d.ap()[:, g * GROUP:(g + 1) * GROUP],
                    in_=n_out)
                if half == 1:
                    ot_tiles.pop(c)

    nc.compile()
    return nc


_NC_CACHE = None


def _prepare_inputs(inputs_np: np.ndarray, params: np.ndarray):
    Ms_xp, bias = host_prep(params)
    X16 = inputs_np.astype(np.float16)
    in_maps = []
    for i in range(N_CORES):
        xt = np.ascontiguousarray(X16[i * ROWS:(i + 1) * ROWS].T)
        in_maps.append({"xt": xt, "ms": Ms_xp})
    return in_maps, bias


def _finish(results, bias):
    out = np.empty((BATCH, N_QUMODES), np.float32)
    biasf = bias.astype(np.float32)[None, :]
    for i, r in enumerate(results):
        nT = r["outT"].astype(np.float32)                    # [128, ROWS]
        out[i * ROWS:(i + 1) * ROWS] = nT.T + biasf
    return out


def run(inputs_np: np.ndarray, params: np.ndarray, trace: bool = False):
    global _NC_CACHE
    if _NC_CACHE is None:
        _NC_CACHE = build_bass()
    nc = _NC_CACHE
    in_maps, bias = _prepare_inputs(inputs_np, params)
    res = run_bass_kernel_spmd(nc, in_maps, core_ids=list(range(N_CORES)),
                               trace=trace)
    out = _finish(res.results, bias)
    return out, res


def kernel(**inputs: np.ndarray) -> np.ndarray:
    X = np.ascontiguousarray(np.asarray(inputs["inputs"], dtype=np.float32))
    params = np.asarray(inputs["params"], dtype=np.float32)
    assert X.shape == (BATCH, N_QUMODES)
    out, _ = run(X, params)
    return out
